# revision 35
# baseline (speedup 1.0000x reference)
"""Trainium2 Bass kernel for CrossAttentionFusion (fp8 DoubleRow, pipelined).

Reference computation (shapes hardcoded):
  B=4, C=256, H=W=128, N=16384, CHUNK=2048, nchunks=8.
  q  = image_features  reshaped to (B, nchunks, CHUNK, C)
  kv = lidar_features  reshaped to (B, nchunks, CHUNK, C)
  per (b, chunk): out = softmax(q @ kv.T / sqrt(C)) @ kv
  output = w0 * image_features + w1 * fused,  w = softmax(modality_weights)

Sharding: the 32 independent (b, chunk) pairs are split 4-per-core across
8 NeuronCores (data parallel over batch x chunk; no communication).

fp8 strategy: Q/KV quantized to fp8e4 (e4m3, max 240) on host; both
matmuls run in MatmulPerfMode.DoubleRow (K=256 per instruction, 2 fp8
MACs/cell/cycle).  P = exp(s/sqrt(C) - 3) is produced in fp8: the -3
bias keeps max P ~17 < 240 and cancels in the softmax ratio because the
row-sum is scaled identically.

Pipeline structure (per core): 16 panel-jobs (4 pairs x 4 q-panels).
mm1 of job i+1 is interleaved between the mm2 tq-groups of job i, so
the exp of job i+1 overlaps job i's mm2 instead of sitting between
mm1 and mm2 on the PE critical path:

  PE:   [mm1 j0 j1][mm2 tq0][mm1 j2 j3][mm2 tq1] ... (job i+1 / job i)
  ACT:  4.5 exp tiles/job (strict alternation with DVE so arrivals are
        evenly spaced) + normalize for tq0/tq2
  DVE:  3.5 exp tiles/job + all recips + fused (psO*r)+w0*q
        scalar_tensor_tensor for tq1/tq3
  GpSimd: residual-add fuse for tq0/tq2 + all DMA triggers

mm2 (q=128p, 257f) = P^T.T @ [KV | 1/w1] over 8 j-pairs; the ones
column is pre-scaled by 1/w1 so recip(rowsum_col) = w1/rowsum.

Startup: kvb rides first on all three DMA queues (sync/scalar/gpsimd),
the first mm1 j-pair runs as 256-col halves on a quarter qb chunk.
Tail: the last q-tile's mm2 runs as two column groups (rowsum side
first) so the final store DMA (~2us latency) overlaps its matmuls.
Engine rates (measured): PE 78.6 TMAC/s fp8-DR (both matmuls, ~111.5us
streaming floor/core), ACT 1.11us / 1024-elem exp tile, DVE 1.23us.
"""

import numpy as np

B, C, H, W = 4, 256, 128, 128
N = H * W
CHUNK = 2048
NCHUNKS = N // CHUNK         # 8
NCORES = 8
PAIRS = B * NCHUNKS          # 32
PPC = PAIRS // NCORES        # 4 pairs (chunks) per core
CT = C // 128                # 2 c-tiles
KT = CHUNK // 128            # 16 k-tiles
QT = CHUNK // 128            # 16 q-tiles
PAN = 512                    # q panel width
NPAN = CHUNK // PAN          # 4 panels
KCS = 272                    # kc tile stride (257 cols used)
JP = KT // 2                 # 8 j-pairs for DoubleRow mm2
NJOBS = PPC * NPAN           # 16 panel-jobs per core
SCALE = 1.0 / float(np.sqrt(C))
EBIAS = -3.0                 # exp bias: keeps max P < fp8e4 max (240)
LOG2E_8 = 8.0 / float(np.log(2.0))          # 2^3 / ln2 for e4m3 int exp
EXPI8_S0 = SCALE * LOG2E_8                  # psS scale
EXPI8_SHIFT = -0.4                          # Schraudolph centering
EXPI8_S1 = 56.0 + EBIAS * LOG2E_8 + EXPI8_SHIFT

_BUILD_CACHE = {}
_EXPI8_OP = None


def _register_expi8():
    """Register a custom DVE op: out = relu(in*C0 + C1), written as int8.

    With C0/C1 set per the Schraudolph trick, the int8 bit pattern IS
    exp(scale*s + bias) in fp8e4 (bitcast), so the DVE can compute exp
    tiles in a single pass and share the softmax work with ACT.
    """
    global _EXPI8_OP
    if _EXPI8_OP is not None:
        return _EXPI8_OP
    import concourse.dve_ops as dve_ops
    from concourse.bass import dve_ver_for
    from concourse.dve_spec import C0, C1, Spec, Src0, lower, relu
    from concourse.dve_uop import DveOpSpec

    name = "EXP_I8_SCHRAUDOLPH"
    for op in dve_ops.OPS:
        if op.name == name:
            _EXPI8_OP = op
            return op

    spec = Spec(
        body=relu(Src0 * C0 + C1),
        reference=lambda in0, in1, c0, c1, c2: np.maximum(
            in0.astype(np.float32) * c0 + c1, 0.0
        ),
    )
    opcode = dve_ops._CUSTOM_DVE_ROW_BASE + len(dve_ops.OPS)
    shas = {}
    for ver in ("v3", "v4"):
        try:
            uops = lower(spec, ver=ver)
            shas[ver] = DveOpSpec(
                name=name, opcode=opcode, uops=uops, rd1_en=False
            ).sha(ver)
        except Exception:
            pass
    op = dve_ops.DveOp(name, spec, subdim=False, uops_sha=shas)
    dve_ops.OPS.append(op)
    dve_ops._SUB_OPCODE_FOR_NAME[name] = opcode
    _EXPI8_OP = op
    return op


def _use_dve(job: int, jj: int) -> bool:
    """Exp engine split (~4.5 ACT / 3.5 DVE per job). ACT is faster per
    tile (1.2 vs 0.96 GHz) and also runs half the psO normalizes; DVE
    runs the recips + the other half of the epilogue. Job 0 has no mm2
    overlap (prologue), so split it evenly to minimize the serial exp
    chain before the first mm2."""
    if job == 0:
        return jj % 2 == 1
    if jj in (1, 3, 5):
        return True
    return jj == 7 and (job % 2 == 1)


def _build(w0: float, w1: float):
    from contextlib import ExitStack

    import concourse.bass as bass
    import concourse.tile as tile
    from concourse import bacc, mybir

    expi8 = _register_expi8()

    f32 = mybir.dt.float32
    bf16 = mybir.dt.bfloat16
    f8 = mybir.dt.float8e4
    i8 = mybir.dt.int8
    DR = mybir.MatmulPerfMode.DoubleRow
    Exp = mybir.ActivationFunctionType.Exp
    Copy = mybir.ActivationFunctionType.Copy
    mult = mybir.AluOpType.mult
    add = mybir.AluOpType.add

    nc = bacc.Bacc("TRN2", target_bir_lowering=False, debug=False)
    qt_d = nc.dram_tensor("qt_sh", (PPC, CHUNK, C), bf16, kind="ExternalInput")
    qb_d = nc.dram_tensor("qb_sh", (PPC, C, CHUNK), f8, kind="ExternalInput")
    kvb_d = nc.dram_tensor("kvb_sh", (PPC, C, CHUNK), f8, kind="ExternalInput")
    kc_d = nc.dram_tensor("kc_sh", (PPC, 128, KT * KCS), f8, kind="ExternalInput")
    out_d = nc.dram_tensor("out_sh", (PPC, CHUNK, C), bf16, kind="ExternalOutput")

    with ExitStack() as ctx:
        tc = ctx.enter_context(tile.TileContext(nc))
        po_qt = ctx.enter_context(tc.tile_pool(name="qt", bufs=2))
        po_qb = ctx.enter_context(tc.tile_pool(name="qb", bufs=2))
        po_kvb = ctx.enter_context(tc.tile_pool(name="kvb", bufs=2))
        po_kc = ctx.enter_context(tc.tile_pool(name="kc", bufs=2))
        po_pt = ctx.enter_context(tc.tile_pool(name="pt", bufs=3))
        po_out = ctx.enter_context(tc.tile_pool(name="outs", bufs=2))
        po_r = ctx.enter_context(tc.tile_pool(name="r", bufs=4))
        po_g = ctx.enter_context(tc.tile_pool(name="g", bufs=6))
        po_psS = ctx.enter_context(tc.tile_pool(name="psS", bufs=3, space="PSUM"))
        po_psO = ctx.enter_context(tc.tile_pool(name="psO", bufs=2, space="PSUM"))
        po_const = ctx.enter_context(tc.tile_pool(name="consts", bufs=1))

        ebias = po_const.tile([128, 1], f32, name="ebias")
        nc.gpsimd.memset(ebias[:], EBIAS)

        pair_tiles = {}
        pair_outs = {}
        job_pts = {}
        jobs = [(p, pan) for p in range(PPC) for pan in range(NPAN)]

        def emit_loads(p, first=False):
            qb = po_qb.tile([128, CT, CHUNK], f8, name="qb")
            kvb = po_kvb.tile([128, CT, CHUNK], f8, name="kvb")
            qt = po_qt.tile([128, QT * C], bf16, name="qt")
            kc = po_kc.tile([128, KT, KCS], f8, name="kc")
            kc2 = kc[:].rearrange("part a b -> part (a b)")
            qt3 = qt[:].rearrange("part (t c) -> part t c", c=C)
            qtd3 = qt_d[p].rearrange("(t part) c -> part t c", part=128)
            if first:
                # Startup: the first mm1 needs kvb j-tile 0 (both ci) +
                # qb panel 0; kc j-pair 0 is needed ~4us in.  Only sync /
                # scalar / gpsimd queues can issue DMA, and triggers cost
                # ~0.7us of issuing-queue time each.  scalar (= ACT) gets
                # just kc[0:2] so it is clear before exp work arrives;
                # kvb+qb interleave j-major on sync (ci=0) / gpsimd (ci=1).
                # Critical chain: the first mm1 j-pair runs as 256-col
                # halves, so it needs only kvb[0:256] + qb[0:256] per ci.
                # Those go first on the three queues (scalar = ACT is
                # clear until the first exp at ~10us); DMA trigger ring
                # throttling (~5 outstanding/queue) punishes more chunks.
                # Startup-critical chain: the first mm1 j-pair runs in
                # 128-col steps, so the very first matmul needs only
                # qb[0:128] (16KB/ci) + kvb[0:256]; each queue's first
                # transfer is tiny and the rest ride behind in need
                # order.
                nc.sync.dma_start(qb[:, 0:1, 0:128], qb_d[p, 0:128, 0:128])
                nc.scalar.dma_start(qb[:, 1:2, 0:128], qb_d[p, 128:256, 0:128])
                nc.sync.dma_start(kvb[:, 0:1, 0:256], kvb_d[p, 0:128, 0:256])
                nc.scalar.dma_start(kvb[:, 1:2, 0:256], kvb_d[p, 128:256, 0:256])
                nc.sync.dma_start(qb[:, 0:1, 128:PAN], qb_d[p, 0:128, 128:PAN])
                nc.scalar.dma_start(qb[:, 1:2, 128:PAN], qb_d[p, 128:256, 128:PAN])
                nc.gpsimd.dma_start(
                    kvb[:, 1:2, 1024:2048], kvb_d[p, 128:256, 1024:2048]
                )
                nc.sync.dma_start(
                    kvb[:, 0:1, 256:1024], kvb_d[p, 0:128, 256:1024]
                )
                nc.scalar.dma_start(
                    kvb[:, 1:2, 256:1024], kvb_d[p, 128:256, 256:1024]
                )
                nc.sync.dma_start(
                    kvb[:, 0:1, 1024:2048], kvb_d[p, 0:128, 1024:2048]
                )
                nc.scalar.dma_start(kc2[:, 0 : 2 * KCS], kc_d[p, :, 0 : 2 * KCS])
                # remaining kc split sync/scalar; qt panel 0 early on
                # gpsimd, the rest of qb / qt after (needed >= 1 job later)
                nc.scalar.dma_start(
                    kc2[:, 2 * KCS : 8 * KCS], kc_d[p, :, 2 * KCS : 8 * KCS]
                )
                nc.gpsimd.dma_start(qt3[:, 0:4, :], qtd3[:, 0:4, :])
                nc.sync.dma_start(
                    kc2[:, 8 * KCS : KT * KCS], kc_d[p, :, 8 * KCS : KT * KCS]
                )
                nc.gpsimd.dma_start(
                    qb[:, 0:1, PAN:CHUNK], qb_d[p, 0:128, PAN:CHUNK]
                )
                nc.gpsimd.dma_start(
                    qb[:, 1:2, PAN:CHUNK], qb_d[p, 128:256, PAN:CHUNK]
                )
                nc.gpsimd.dma_start(qt3[:, 4:QT, :], qtd3[:, 4:QT, :])
            else:
                # Steady state: pair p+1 loads issue a full pair (~27us)
                # ahead; keep triggers off the busy ACT/DVE queues.
                for ci in range(CT):
                    nc.sync.dma_start(
                        kvb[:, ci : ci + 1, :],
                        kvb_d[p, ci * 128 : (ci + 1) * 128, :],
                    )
                    nc.gpsimd.dma_start(
                        qb[:, ci : ci + 1, :],
                        qb_d[p, ci * 128 : (ci + 1) * 128, :],
                    )
                nc.sync.dma_start(kc2[:], kc_d[p, :, :])
                nc.gpsimd.dma_start(qt3[:], qtd3[:])
            pair_tiles[p] = (qb, kvb, qt, kc)

        def emit_mm1_seg(job, jj):
            """One j-pair of mm1 for `job` + its exp into the pt tile."""
            p, pan = jobs[job]
            qb, kvb, qt, kc = pair_tiles[p]
            if jj == 0:
                job_pts[job] = po_pt.tile([128, KT, PAN], f8, name="pt")
            pt = job_pts[job]
            psS = po_psS.tile([128, 2, PAN], f32, name="psS")
            # Job 0's first j-pair is split into col steps so the first
            # matmul starts on a 16KB/ci slice of the qb panel-0 DMA.
            halves = (
                ((0, 128), (128, 256), (256, PAN))
                if job == 0 and jj == 0
                else ((0, PAN),)
            )
            for q0, q1 in halves:
                for i in range(2):
                    j = 2 * jj + i
                    nc.tensor.matmul(
                        psS[:, i : i + 1, q0:q1],
                        lhsT=kvb[:, :, j * 128 : (j + 1) * 128],
                        rhs=qb[:, :, pan * PAN + q0 : pan * PAN + q1],
                        start=True,
                        stop=True,
                        perf_mode=DR,
                    )
            dst = pt[:, 2 * jj : 2 * jj + 2, :]
            if _use_dve(job, jj):
                nc.vector._custom_dve(
                    expi8,
                    out=dst.bitcast(i8),
                    in0=psS[:],
                    s0=EXPI8_S0,
                    s1=EXPI8_S1,
                )
            else:
                nc.scalar.activation(
                    dst, psS[:], Exp, bias=ebias[:], scale=SCALE
                )

        job_psO = {}

        def emit_mm2_half(job, tq, half, c_lo=0, c_hi=C + 1, alloc=None):
            """Half a q-tile of mm2 (4 of 8 accumulating jp matmuls)."""
            p, pan = jobs[job]
            qb, kvb, qt, kc = pair_tiles[p]
            pt = job_pts[job]
            if alloc is None:
                alloc = half == 0
            if alloc:
                job_psO[job] = po_psO.tile([128, 512], f32, name="psO")
            psO = job_psO[job]
            for jp in range(4 * half, 4 * half + 4):
                nc.tensor.matmul(
                    psO[:, c_lo:c_hi],
                    lhsT=pt[:, 2 * jp : 2 * jp + 2, tq * 128 : (tq + 1) * 128],
                    rhs=kc[:, 2 * jp : 2 * jp + 2, c_lo:c_hi],
                    start=(jp == 0),
                    stop=(jp == JP - 1),
                    perf_mode=DR,
                )

        def emit_epilogue(job, tq):
            """Normalize + residual-add + stores for one q-tile.  recip
            always on DVE; the psO drain alternates between a DVE
            scalar_tensor_tensor and ACT-normalize + GpSimd-fuse so both
            engines stay below ~87% with their exp share."""
            p, pan = jobs[job]
            qb, kvb, qt, kc = pair_tiles[p]
            outs = pair_outs[p]
            t = pan * 4 + tq
            psO = job_psO[job]
            r = po_r.tile([128, 1], f32, name="r")
            nc.vector.reciprocal(r[:], psO[:, C : C + 1])
            # out = (psO * w1/rowsum) + w0*q  (qt is pre-scaled by w0 on
            # host; kc's ones column by 1/w1).  The last job drains all
            # four tiles on the DVE in column halves with eager
            # half-stores: the out-store DMA has ~2-3us latency, so the
            # tail is dominated by how early the final store triggers.
            if job == NJOBS - 1:
                o3 = outs[:].rearrange("part (tt c) -> part tt c", c=C)
                od3 = out_d[p].rearrange("(tt part) c -> part tt c", part=128)
                engs = (nc.sync, nc.gpsimd) if tq < 2 else (nc.scalar, nc.sync)
                for h, eng in ((0, engs[0]), (1, engs[1])):
                    c0, c1 = h * 128, h * 128 + 128
                    nc.vector.scalar_tensor_tensor(
                        outs[:, t * C + c0 : t * C + c1],
                        psO[:, c0:c1],
                        r[:],
                        qt[:, t * C + c0 : t * C + c1],
                        op0=mult,
                        op1=add,
                    )
                    eng.dma_start(
                        od3[:, t : t + 1, c0:c1], o3[:, t : t + 1, c0:c1]
                    )
                return
            if tq % 2 == 1:
                nc.vector.scalar_tensor_tensor(
                    outs[:, t * C : (t + 1) * C],
                    psO[:, 0:C],
                    r[:],
                    qt[:, t * C : (t + 1) * C],
                    op0=mult,
                    op1=add,
                )
            else:
                g = po_g.tile([128, C], bf16, name="g")
                nc.scalar.activation(
                    g[:], psO[:, 0:C], Copy, bias=0.0, scale=r[:]
                )
                nc.gpsimd.tensor_tensor(
                    outs[:, t * C : (t + 1) * C],
                    qt[:, t * C : (t + 1) * C],
                    g[:],
                    op=add,
                )
            if tq == 3:
                o3 = outs[:].rearrange("part (tt c) -> part tt c", c=C)
                od3 = out_d[p].rearrange("(tt part) c -> part tt c", part=128)
                nc.gpsimd.dma_start(
                    od3[:, t - 3 : t + 1, :], o3[:, t - 3 : t + 1, :]
                )

        emit_loads(0, first=True)
        pair_outs[0] = po_out.tile([128, QT * C], bf16, name="outs")
        for jj in range(JP):
            emit_mm1_seg(0, jj)
        for i in range(NJOBS):
            p, pan = jobs[i]
            if pan == 0 and p + 1 < PPC:
                emit_loads(p + 1)
                pair_outs[p + 1] = po_out.tile([128, QT * C], bf16, name="outs")
            for s in range(4):
                if i + 1 < NJOBS:
                    emit_mm1_seg(i + 1, 2 * s)
                    emit_mm1_seg(i + 1, 2 * s + 1)
                if i == NJOBS - 1 and s == 3:
                    # Very last q-tile: compute cols [128:257] (incl the
                    # rowsum col) first so its epilogue + store overlap
                    # the cols [0:128] matmuls -- the final store's
                    # ~2.5us DMA latency dominates the kernel tail.
                    p_, pan_ = jobs[i]
                    _, _, qt_, _ = pair_tiles[p_]
                    outs_ = pair_outs[p_]
                    t_ = pan_ * 4 + s
                    o3 = outs_[:].rearrange("part (tt c) -> part tt c", c=C)
                    od3 = out_d[p_].rearrange(
                        "(tt part) c -> part tt c", part=128
                    )
                    emit_mm2_half(i, s, 0, c_lo=128, alloc=True)
                    emit_mm2_half(i, s, 1, c_lo=128, alloc=False)
                    psO_ = job_psO[i]
                    r_ = po_r.tile([128, 1], f32, name="r")
                    nc.vector.reciprocal(r_[:], psO_[:, C : C + 1])
                    nc.vector.scalar_tensor_tensor(
                        outs_[:, t_ * C + 128 : t_ * C + 256],
                        psO_[:, 128:256],
                        r_[:],
                        qt_[:, t_ * C + 128 : t_ * C + 256],
                        op0=mult,
                        op1=add,
                    )
                    nc.scalar.dma_start(
                        od3[:, t_ : t_ + 1, 128:256],
                        o3[:, t_ : t_ + 1, 128:256],
                    )
                    emit_mm2_half(i, s, 0, c_hi=128, alloc=False)
                    emit_mm2_half(i, s, 1, c_hi=128, alloc=False)
                    nc.vector.scalar_tensor_tensor(
                        outs_[:, t_ * C : t_ * C + 128],
                        psO_[:, 0:128],
                        r_[:],
                        qt_[:, t_ * C : t_ * C + 128],
                        op0=mult,
                        op1=add,
                    )
                    # quarter stores on two idle queues: the final DMA's
                    # completion bounds kernel end
                    nc.sync.dma_start(
                        od3[:, t_ : t_ + 1, 0:64], o3[:, t_ : t_ + 1, 0:64]
                    )
                    nc.gpsimd.dma_start(
                        od3[:, t_ : t_ + 1, 64:128], o3[:, t_ : t_ + 1, 64:128]
                    )
                else:
                    emit_mm2_half(i, s, 0)
                    emit_mm2_half(i, s, 1)
                    emit_epilogue(i, s)

    nc.compile()
    return nc


def _get_nc(w0: float, w1: float):
    key = (round(float(w0), 9), round(float(w1), 9))
    if key not in _BUILD_CACHE:
        _BUILD_CACHE[key] = _build(*key)
    return _BUILD_CACHE[key]


def _pairs(arr: np.ndarray) -> np.ndarray:
    # (B, C, H, W) -> (PAIRS, C, CHUNK)
    return (
        arr.reshape(B, C, NCHUNKS, CHUNK)
        .transpose(0, 2, 1, 3)
        .reshape(PAIRS, C, CHUNK)
    )


def _unshard_qc(per_core: list[np.ndarray]) -> np.ndarray:
    # per-core (PPC, CHUNK, C) in (q, c) layout -> (B, C, H, W)
    pairs = np.concatenate(per_core, axis=0)  # (PAIRS, CHUNK, C)
    return np.ascontiguousarray(
        pairs.reshape(B, NCHUNKS, CHUNK, C)
        .transpose(0, 3, 1, 2)
        .reshape(B, C, H, W)
    )


def run(lidar_features, image_features, modality_weights, trace=False):
    import ml_dtypes

    from concourse import bass_utils

    f8 = ml_dtypes.float8_e4m3

    mw = np.asarray(modality_weights, dtype=np.float64)
    e = np.exp(mw - mw.max())
    wsm = e / e.sum()
    w0, w1 = float(wsm[0]), float(wsm[1])

    nc = _get_nc(w0, w1)

    qp = _pairs(np.asarray(image_features, dtype=np.float32))
    kvp = _pairs(np.asarray(lidar_features, dtype=np.float32))
    qpb = qp.astype(f8)
    kvpb = kvp.astype(f8)
    # w0*Q in (q, c) layout (bf16) for the fuse term (pre-scaled on host so
    # the device fuse is a single add)
    qpt = np.ascontiguousarray(qp.transpose(0, 2, 1) * w0).astype(
        ml_dtypes.bfloat16
    )
    # pre-packed (w1*KV) (k, c) tiles + ones column, exactly the kc SBUF
    # layout: psO = P @ (w1*KV) | rowsum, so g = psO * recip(rowsum) needs
    # no extra w1 multiply.
    kcp = np.zeros((PAIRS, 128, KT, KCS), dtype=f8)
    kvw = (kvp * w1).astype(f8)
    # kc[pair, k_in_tile, j, c] = w1 * KV[pair, c, j*128 + k_in_tile]
    kcp[:, :, :, 0:C] = kvw.reshape(PAIRS, C, KT, 128).transpose(0, 3, 2, 1)
    kcp[:, :, :, C] = 1.0
    kcp = kcp.reshape(PAIRS, 128, KT * KCS)
    in_maps = [
        {
            "qt_sh": np.ascontiguousarray(qpt[i * PPC : (i + 1) * PPC]),
            "qb_sh": np.ascontiguousarray(qpb[i * PPC : (i + 1) * PPC]),
            "kvb_sh": np.ascontiguousarray(kvpb[i * PPC : (i + 1) * PPC]),
            "kc_sh": np.ascontiguousarray(kcp[i * PPC : (i + 1) * PPC]),
        }
        for i in range(NCORES)
    ]
    res = bass_utils.run_bass_kernel_spmd(
        nc, in_maps, core_ids=list(range(NCORES)), trace=trace
    )
    out = _unshard_qc(
        [res.results[i]["out_sh"].astype(np.float32) for i in range(NCORES)]
    )
    return out, res


def kernel(lidar_features, image_features, modality_weights) -> np.ndarray:
    out, _ = run(lidar_features, image_features, modality_weights, trace=False)
    return out


# revision 37
# speedup vs baseline: 1.0079x; 1.0079x over previous
"""Trainium2 Bass kernel for CrossAttentionFusion (fp8 DoubleRow, pipelined).

Reference computation (shapes hardcoded):
  B=4, C=256, H=W=128, N=16384, CHUNK=2048, nchunks=8.
  q  = image_features  reshaped to (B, nchunks, CHUNK, C)
  kv = lidar_features  reshaped to (B, nchunks, CHUNK, C)
  per (b, chunk): out = softmax(q @ kv.T / sqrt(C)) @ kv
  output = w0 * image_features + w1 * fused,  w = softmax(modality_weights)

Sharding: the 32 independent (b, chunk) pairs are split 4-per-core across
8 NeuronCores (data parallel over batch x chunk; no communication).

fp8 strategy: Q/KV quantized to fp8e4 (e4m3, max 240) on host; both
matmuls run in MatmulPerfMode.DoubleRow (K=256 per instruction, 2 fp8
MACs/cell/cycle).  P = exp(s/sqrt(C) - 3) is produced in fp8: the -3
bias keeps max P ~17 < 240 and cancels in the softmax ratio because the
row-sum is scaled identically.

Pipeline structure (per core): 16 panel-jobs (4 pairs x 4 q-panels).
mm1 of job i+1 is interleaved between the mm2 tq-groups of job i, so
the exp of job i+1 overlaps job i's mm2 instead of sitting between
mm1 and mm2 on the PE critical path:

  PE:   [mm1 j0 j1][mm2 tq0][mm1 j2 j3][mm2 tq1] ... (job i+1 / job i)
  ACT:  4.5 exp tiles/job (strict alternation with DVE so arrivals are
        evenly spaced) + normalize for tq0/tq2
  DVE:  3.5 exp tiles/job + all recips + fused (psO*r)+w0*q
        scalar_tensor_tensor for tq1/tq3
  GpSimd: residual-add fuse for tq0/tq2 + all DMA triggers

mm2 (q=128p, 257f) = P^T.T @ [KV | 1/w1] over 8 j-pairs; the ones
column is pre-scaled by 1/w1 so recip(rowsum_col) = w1/rowsum.

Startup: kvb rides first on all three DMA queues (sync/scalar/gpsimd),
the first mm1 j-pair runs as 256-col halves on a quarter qb chunk.
Tail: the last q-tile's mm2 runs as two column groups (rowsum side
first) so the final store DMA (~2us latency) overlaps its matmuls.
Engine rates (measured): PE 78.6 TMAC/s fp8-DR (both matmuls, ~111.5us
streaming floor/core), ACT 1.11us / 1024-elem exp tile, DVE 1.23us.
"""

import numpy as np

B, C, H, W = 4, 256, 128, 128
N = H * W
CHUNK = 2048
NCHUNKS = N // CHUNK         # 8
NCORES = 8
PAIRS = B * NCHUNKS          # 32
PPC = PAIRS // NCORES        # 4 pairs (chunks) per core
CT = C // 128                # 2 c-tiles
KT = CHUNK // 128            # 16 k-tiles
QT = CHUNK // 128            # 16 q-tiles
PAN = 512                    # q panel width
NPAN = CHUNK // PAN          # 4 panels
KCS = 272                    # kc tile stride (257 cols used)
JP = KT // 2                 # 8 j-pairs for DoubleRow mm2
NJOBS = PPC * NPAN           # 16 panel-jobs per core
SCALE = 1.0 / float(np.sqrt(C))
EBIAS = -3.0                 # exp bias: keeps max P < fp8e4 max (240)
LOG2E_8 = 8.0 / float(np.log(2.0))          # 2^3 / ln2 for e4m3 int exp
EXPI8_S0 = SCALE * LOG2E_8                  # psS scale
EXPI8_SHIFT = -0.4                          # Schraudolph centering
EXPI8_S1 = 56.0 + EBIAS * LOG2E_8 + EXPI8_SHIFT

_BUILD_CACHE = {}
_EXPI8_OP = None


def _register_expi8():
    """Register a custom DVE op: out = relu(in*C0 + C1), written as int8.

    With C0/C1 set per the Schraudolph trick, the int8 bit pattern IS
    exp(scale*s + bias) in fp8e4 (bitcast), so the DVE can compute exp
    tiles in a single pass and share the softmax work with ACT.
    """
    global _EXPI8_OP
    if _EXPI8_OP is not None:
        return _EXPI8_OP
    import concourse.dve_ops as dve_ops
    from concourse.bass import dve_ver_for
    from concourse.dve_spec import C0, C1, Spec, Src0, lower, relu
    from concourse.dve_uop import DveOpSpec

    name = "EXP_I8_SCHRAUDOLPH"
    for op in dve_ops.OPS:
        if op.name == name:
            _EXPI8_OP = op
            return op

    spec = Spec(
        body=relu(Src0 * C0 + C1),
        reference=lambda in0, in1, c0, c1, c2: np.maximum(
            in0.astype(np.float32) * c0 + c1, 0.0
        ),
    )
    opcode = dve_ops._CUSTOM_DVE_ROW_BASE + len(dve_ops.OPS)
    shas = {}
    for ver in ("v3", "v4"):
        try:
            uops = lower(spec, ver=ver)
            shas[ver] = DveOpSpec(
                name=name, opcode=opcode, uops=uops, rd1_en=False
            ).sha(ver)
        except Exception:
            pass
    op = dve_ops.DveOp(name, spec, subdim=False, uops_sha=shas)
    dve_ops.OPS.append(op)
    dve_ops._SUB_OPCODE_FOR_NAME[name] = opcode
    _EXPI8_OP = op
    return op


def _use_dve(job: int, jj: int) -> bool:
    """Exp engine split (~4.5 ACT / 3.5 DVE per job). ACT is faster per
    tile (1.2 vs 0.96 GHz) and also runs half the psO normalizes; DVE
    runs the recips + the other half of the epilogue. Job 0 has no mm2
    overlap (prologue), so split it evenly to minimize the serial exp
    chain before the first mm2."""
    if job == 0:
        return jj % 2 == 1
    if jj in (1, 3, 5):
        return True
    return jj == 7 and (job % 2 == 1)


def _build(w0: float, w1: float):
    from contextlib import ExitStack

    import concourse.bass as bass
    import concourse.tile as tile
    from concourse import bacc, mybir

    expi8 = _register_expi8()

    f32 = mybir.dt.float32
    bf16 = mybir.dt.bfloat16
    f8 = mybir.dt.float8e4
    i8 = mybir.dt.int8
    DR = mybir.MatmulPerfMode.DoubleRow
    Exp = mybir.ActivationFunctionType.Exp
    Copy = mybir.ActivationFunctionType.Copy
    mult = mybir.AluOpType.mult
    add = mybir.AluOpType.add

    nc = bacc.Bacc("TRN2", target_bir_lowering=False, debug=False)
    qt_d = nc.dram_tensor("qt_sh", (PPC, CHUNK, C), bf16, kind="ExternalInput")
    qb_d = nc.dram_tensor("qb_sh", (PPC, C, CHUNK), f8, kind="ExternalInput")
    kvb_d = nc.dram_tensor("kvb_sh", (PPC, C, CHUNK), f8, kind="ExternalInput")
    kc_d = nc.dram_tensor("kc_sh", (PPC, 128, KT * KCS), f8, kind="ExternalInput")
    out_d = nc.dram_tensor("out_sh", (PPC, CHUNK, C), bf16, kind="ExternalOutput")

    with ExitStack() as ctx:
        tc = ctx.enter_context(tile.TileContext(nc))
        po_qt = ctx.enter_context(tc.tile_pool(name="qt", bufs=2))
        po_qb = ctx.enter_context(tc.tile_pool(name="qb", bufs=2))
        po_kvb = ctx.enter_context(tc.tile_pool(name="kvb", bufs=2))
        po_kc = ctx.enter_context(tc.tile_pool(name="kc", bufs=2))
        po_pt = ctx.enter_context(tc.tile_pool(name="pt", bufs=3))
        po_out = ctx.enter_context(tc.tile_pool(name="outs", bufs=2))
        po_r = ctx.enter_context(tc.tile_pool(name="r", bufs=4))
        po_g = ctx.enter_context(tc.tile_pool(name="g", bufs=6))
        po_psS = ctx.enter_context(tc.tile_pool(name="psS", bufs=3, space="PSUM"))
        po_psO = ctx.enter_context(tc.tile_pool(name="psO", bufs=2, space="PSUM"))
        po_const = ctx.enter_context(tc.tile_pool(name="consts", bufs=1))

        ebias = po_const.tile([128, 1], f32, name="ebias")
        nc.gpsimd.memset(ebias[:], EBIAS)

        pair_tiles = {}
        pair_outs = {}
        job_pts = {}
        jobs = [(p, pan) for p in range(PPC) for pan in range(NPAN)]

        def emit_loads(p, first=False):
            qb = po_qb.tile([128, CT, CHUNK], f8, name="qb")
            kvb = po_kvb.tile([128, CT, CHUNK], f8, name="kvb")
            qt = po_qt.tile([128, QT * C], bf16, name="qt")
            kc = po_kc.tile([128, KT, KCS], f8, name="kc")
            kc2 = kc[:].rearrange("part a b -> part (a b)")
            qt3 = qt[:].rearrange("part (t c) -> part t c", c=C)
            qtd3 = qt_d[p].rearrange("(t part) c -> part t c", part=128)
            if first:
                # Startup: the first mm1 needs kvb j-tile 0 (both ci) +
                # qb panel 0; kc j-pair 0 is needed ~4us in.  Only sync /
                # scalar / gpsimd queues can issue DMA, and triggers cost
                # ~0.7us of issuing-queue time each.  scalar (= ACT) gets
                # just kc[0:2] so it is clear before exp work arrives;
                # kvb+qb interleave j-major on sync (ci=0) / gpsimd (ci=1).
                # Critical chain: the first mm1 j-pair runs as 256-col
                # halves, so it needs only kvb[0:256] + qb[0:256] per ci.
                # Those go first on the three queues (scalar = ACT is
                # clear until the first exp at ~10us); DMA trigger ring
                # throttling (~5 outstanding/queue) punishes more chunks.
                # kvb (both ci) is the startup-critical mass: mm1 of job
                # 0 sweeps all 16 j-tiles in ~7us, so kvb rides early on
                # all three queues; qb panel 0 halves lead for the
                # (column-split) first matmul.
                nc.sync.dma_start(qb[:, 0:1, 0:256], qb_d[p, 0:128, 0:256])
                nc.scalar.dma_start(qb[:, 1:2, 0:256], qb_d[p, 128:256, 0:256])
                nc.sync.dma_start(kvb[:, 0:1, 0:256], kvb_d[p, 0:128, 0:256])
                nc.scalar.dma_start(kvb[:, 1:2, 0:256], kvb_d[p, 128:256, 0:256])
                nc.gpsimd.dma_start(
                    kvb[:, 1:2, 1024:2048], kvb_d[p, 128:256, 1024:2048]
                )
                nc.sync.dma_start(
                    kvb[:, 0:1, 256:1024], kvb_d[p, 0:128, 256:1024]
                )
                nc.scalar.dma_start(
                    kvb[:, 1:2, 256:1024], kvb_d[p, 128:256, 256:1024]
                )
                nc.gpsimd.dma_start(qb[:, 0:1, 256:PAN], qb_d[p, 0:128, 256:PAN])
                nc.gpsimd.dma_start(qb[:, 1:2, 256:PAN], qb_d[p, 128:256, 256:PAN])
                nc.sync.dma_start(
                    kvb[:, 0:1, 1024:2048], kvb_d[p, 0:128, 1024:2048]
                )
                nc.scalar.dma_start(kc2[:, 0 : 2 * KCS], kc_d[p, :, 0 : 2 * KCS])
                # remaining kc split sync/scalar; qt panel 0 early on
                # gpsimd, the rest of qb / qt after (needed >= 1 job later)
                nc.scalar.dma_start(
                    kc2[:, 2 * KCS : 8 * KCS], kc_d[p, :, 2 * KCS : 8 * KCS]
                )
                nc.gpsimd.dma_start(qt3[:, 0:4, :], qtd3[:, 0:4, :])
                nc.sync.dma_start(
                    kc2[:, 8 * KCS : KT * KCS], kc_d[p, :, 8 * KCS : KT * KCS]
                )
                nc.gpsimd.dma_start(
                    qb[:, 0:1, PAN:CHUNK], qb_d[p, 0:128, PAN:CHUNK]
                )
                nc.gpsimd.dma_start(
                    qb[:, 1:2, PAN:CHUNK], qb_d[p, 128:256, PAN:CHUNK]
                )
                nc.gpsimd.dma_start(qt3[:, 4:QT, :], qtd3[:, 4:QT, :])
            else:
                # Steady state: pair p+1 loads issue a full pair (~27us)
                # ahead; keep triggers off the busy ACT/DVE queues.
                for ci in range(CT):
                    nc.sync.dma_start(
                        kvb[:, ci : ci + 1, :],
                        kvb_d[p, ci * 128 : (ci + 1) * 128, :],
                    )
                    nc.gpsimd.dma_start(
                        qb[:, ci : ci + 1, :],
                        qb_d[p, ci * 128 : (ci + 1) * 128, :],
                    )
                nc.sync.dma_start(kc2[:], kc_d[p, :, :])
                nc.gpsimd.dma_start(qt3[:], qtd3[:])
            pair_tiles[p] = (qb, kvb, qt, kc)

        def emit_mm1_seg(job, jj):
            """One j-pair of mm1 for `job` + its exp into the pt tile."""
            p, pan = jobs[job]
            qb, kvb, qt, kc = pair_tiles[p]
            if jj == 0:
                job_pts[job] = po_pt.tile([128, KT, PAN], f8, name="pt")
            pt = job_pts[job]
            psS = po_psS.tile([128, 2, PAN], f32, name="psS")
            # Job 0's first j-pair is split into 256-col halves so the
            # first matmul starts on a quarter of the qb panel-0 DMA.
            halves = ((0, 256), (256, PAN)) if job == 0 and jj == 0 else ((0, PAN),)
            for q0, q1 in halves:
                for i in range(2):
                    j = 2 * jj + i
                    nc.tensor.matmul(
                        psS[:, i : i + 1, q0:q1],
                        lhsT=kvb[:, :, j * 128 : (j + 1) * 128],
                        rhs=qb[:, :, pan * PAN + q0 : pan * PAN + q1],
                        start=True,
                        stop=True,
                        perf_mode=DR,
                    )
            dst = pt[:, 2 * jj : 2 * jj + 2, :]
            if _use_dve(job, jj):
                nc.vector._custom_dve(
                    expi8,
                    out=dst.bitcast(i8),
                    in0=psS[:],
                    s0=EXPI8_S0,
                    s1=EXPI8_S1,
                )
            else:
                nc.scalar.activation(
                    dst, psS[:], Exp, bias=ebias[:], scale=SCALE
                )

        job_psO = {}

        def emit_mm2_half(job, tq, half, c_lo=0, c_hi=C + 1, alloc=None):
            """Half a q-tile of mm2 (4 of 8 accumulating jp matmuls)."""
            p, pan = jobs[job]
            qb, kvb, qt, kc = pair_tiles[p]
            pt = job_pts[job]
            if alloc is None:
                alloc = half == 0
            if alloc:
                job_psO[job] = po_psO.tile([128, 512], f32, name="psO")
            psO = job_psO[job]
            for jp in range(4 * half, 4 * half + 4):
                nc.tensor.matmul(
                    psO[:, c_lo:c_hi],
                    lhsT=pt[:, 2 * jp : 2 * jp + 2, tq * 128 : (tq + 1) * 128],
                    rhs=kc[:, 2 * jp : 2 * jp + 2, c_lo:c_hi],
                    start=(jp == 0),
                    stop=(jp == JP - 1),
                    perf_mode=DR,
                )

        def emit_epilogue(job, tq):
            """Normalize + residual-add + stores for one q-tile.  recip
            always on DVE; the psO drain alternates between a DVE
            scalar_tensor_tensor and ACT-normalize + GpSimd-fuse so both
            engines stay below ~87% with their exp share."""
            p, pan = jobs[job]
            qb, kvb, qt, kc = pair_tiles[p]
            outs = pair_outs[p]
            t = pan * 4 + tq
            psO = job_psO[job]
            r = po_r.tile([128, 1], f32, name="r")
            nc.vector.reciprocal(r[:], psO[:, C : C + 1])
            # out = (psO * w1/rowsum) + w0*q  (qt is pre-scaled by w0 on
            # host; kc's ones column by 1/w1).  The last job drains all
            # four tiles on the DVE in column halves with eager
            # half-stores: the out-store DMA has ~2-3us latency, so the
            # tail is dominated by how early the final store triggers.
            if job == NJOBS - 1:
                o3 = outs[:].rearrange("part (tt c) -> part tt c", c=C)
                od3 = out_d[p].rearrange("(tt part) c -> part tt c", part=128)
                engs = (nc.sync, nc.gpsimd) if tq < 2 else (nc.scalar, nc.sync)
                for h, eng in ((0, engs[0]), (1, engs[1])):
                    c0, c1 = h * 128, h * 128 + 128
                    nc.vector.scalar_tensor_tensor(
                        outs[:, t * C + c0 : t * C + c1],
                        psO[:, c0:c1],
                        r[:],
                        qt[:, t * C + c0 : t * C + c1],
                        op0=mult,
                        op1=add,
                    )
                    eng.dma_start(
                        od3[:, t : t + 1, c0:c1], o3[:, t : t + 1, c0:c1]
                    )
                return
            if tq % 2 == 1:
                nc.vector.scalar_tensor_tensor(
                    outs[:, t * C : (t + 1) * C],
                    psO[:, 0:C],
                    r[:],
                    qt[:, t * C : (t + 1) * C],
                    op0=mult,
                    op1=add,
                )
            else:
                g = po_g.tile([128, C], bf16, name="g")
                nc.scalar.activation(
                    g[:], psO[:, 0:C], Copy, bias=0.0, scale=r[:]
                )
                nc.gpsimd.tensor_tensor(
                    outs[:, t * C : (t + 1) * C],
                    qt[:, t * C : (t + 1) * C],
                    g[:],
                    op=add,
                )
            if tq == 3:
                o3 = outs[:].rearrange("part (tt c) -> part tt c", c=C)
                od3 = out_d[p].rearrange("(tt part) c -> part tt c", part=128)
                nc.gpsimd.dma_start(
                    od3[:, t - 3 : t + 1, :], o3[:, t - 3 : t + 1, :]
                )

        emit_loads(0, first=True)
        pair_outs[0] = po_out.tile([128, QT * C], bf16, name="outs")
        for jj in range(JP):
            emit_mm1_seg(0, jj)
        for i in range(NJOBS):
            p, pan = jobs[i]
            if pan == 0 and p + 1 < PPC:
                emit_loads(p + 1)
                pair_outs[p + 1] = po_out.tile([128, QT * C], bf16, name="outs")
            for s in range(4):
                if i + 1 < NJOBS:
                    emit_mm1_seg(i + 1, 2 * s)
                    emit_mm1_seg(i + 1, 2 * s + 1)
                if i == NJOBS - 1 and s == 3:
                    # Very last q-tile: compute cols [128:257] (incl the
                    # rowsum col) first so its epilogue + store overlap
                    # the cols [0:128] matmuls -- the final store's
                    # ~2.5us DMA latency dominates the kernel tail.
                    p_, pan_ = jobs[i]
                    _, _, qt_, _ = pair_tiles[p_]
                    outs_ = pair_outs[p_]
                    t_ = pan_ * 4 + s
                    o3 = outs_[:].rearrange("part (tt c) -> part tt c", c=C)
                    od3 = out_d[p_].rearrange(
                        "(tt part) c -> part tt c", part=128
                    )
                    emit_mm2_half(i, s, 0, c_lo=128, alloc=True)
                    emit_mm2_half(i, s, 1, c_lo=128, alloc=False)
                    psO_ = job_psO[i]
                    r_ = po_r.tile([128, 1], f32, name="r")
                    nc.vector.reciprocal(r_[:], psO_[:, C : C + 1])
                    nc.vector.scalar_tensor_tensor(
                        outs_[:, t_ * C + 128 : t_ * C + 256],
                        psO_[:, 128:256],
                        r_[:],
                        qt_[:, t_ * C + 128 : t_ * C + 256],
                        op0=mult,
                        op1=add,
                    )
                    nc.scalar.dma_start(
                        od3[:, t_ : t_ + 1, 128:256],
                        o3[:, t_ : t_ + 1, 128:256],
                    )
                    emit_mm2_half(i, s, 0, c_hi=128, alloc=False)
                    emit_mm2_half(i, s, 1, c_hi=128, alloc=False)
                    nc.vector.scalar_tensor_tensor(
                        outs_[:, t_ * C : t_ * C + 128],
                        psO_[:, 0:128],
                        r_[:],
                        qt_[:, t_ * C : t_ * C + 128],
                        op0=mult,
                        op1=add,
                    )
                    # quarter stores on two idle queues: the final DMA's
                    # completion bounds kernel end
                    nc.sync.dma_start(
                        od3[:, t_ : t_ + 1, 0:64], o3[:, t_ : t_ + 1, 0:64]
                    )
                    nc.gpsimd.dma_start(
                        od3[:, t_ : t_ + 1, 64:128], o3[:, t_ : t_ + 1, 64:128]
                    )
                else:
                    emit_mm2_half(i, s, 0)
                    emit_mm2_half(i, s, 1)
                    emit_epilogue(i, s)

    nc.compile()
    return nc


def _get_nc(w0: float, w1: float):
    key = (round(float(w0), 9), round(float(w1), 9))
    if key not in _BUILD_CACHE:
        _BUILD_CACHE[key] = _build(*key)
    return _BUILD_CACHE[key]


def _pairs(arr: np.ndarray) -> np.ndarray:
    # (B, C, H, W) -> (PAIRS, C, CHUNK)
    return (
        arr.reshape(B, C, NCHUNKS, CHUNK)
        .transpose(0, 2, 1, 3)
        .reshape(PAIRS, C, CHUNK)
    )


def _unshard_qc(per_core: list[np.ndarray]) -> np.ndarray:
    # per-core (PPC, CHUNK, C) in (q, c) layout -> (B, C, H, W)
    pairs = np.concatenate(per_core, axis=0)  # (PAIRS, CHUNK, C)
    return np.ascontiguousarray(
        pairs.reshape(B, NCHUNKS, CHUNK, C)
        .transpose(0, 3, 1, 2)
        .reshape(B, C, H, W)
    )


def run(lidar_features, image_features, modality_weights, trace=False):
    import ml_dtypes

    from concourse import bass_utils

    f8 = ml_dtypes.float8_e4m3

    mw = np.asarray(modality_weights, dtype=np.float64)
    e = np.exp(mw - mw.max())
    wsm = e / e.sum()
    w0, w1 = float(wsm[0]), float(wsm[1])

    nc = _get_nc(w0, w1)

    qp = _pairs(np.asarray(image_features, dtype=np.float32))
    kvp = _pairs(np.asarray(lidar_features, dtype=np.float32))
    qpb = qp.astype(f8)
    kvpb = kvp.astype(f8)
    # w0*Q in (q, c) layout (bf16) for the fuse term (pre-scaled on host so
    # the device fuse is a single add)
    qpt = np.ascontiguousarray(qp.transpose(0, 2, 1) * w0).astype(
        ml_dtypes.bfloat16
    )
    # pre-packed (w1*KV) (k, c) tiles + ones column, exactly the kc SBUF
    # layout: psO = P @ (w1*KV) | rowsum, so g = psO * recip(rowsum) needs
    # no extra w1 multiply.
    kcp = np.zeros((PAIRS, 128, KT, KCS), dtype=f8)
    kvw = (kvp * w1).astype(f8)
    # kc[pair, k_in_tile, j, c] = w1 * KV[pair, c, j*128 + k_in_tile]
    kcp[:, :, :, 0:C] = kvw.reshape(PAIRS, C, KT, 128).transpose(0, 3, 2, 1)
    kcp[:, :, :, C] = 1.0
    kcp = kcp.reshape(PAIRS, 128, KT * KCS)
    in_maps = [
        {
            "qt_sh": np.ascontiguousarray(qpt[i * PPC : (i + 1) * PPC]),
            "qb_sh": np.ascontiguousarray(qpb[i * PPC : (i + 1) * PPC]),
            "kvb_sh": np.ascontiguousarray(kvpb[i * PPC : (i + 1) * PPC]),
            "kc_sh": np.ascontiguousarray(kcp[i * PPC : (i + 1) * PPC]),
        }
        for i in range(NCORES)
    ]
    res = bass_utils.run_bass_kernel_spmd(
        nc, in_maps, core_ids=list(range(NCORES)), trace=trace
    )
    out = _unshard_qc(
        [res.results[i]["out_sh"].astype(np.float32) for i in range(NCORES)]
    )
    return out, res


def kernel(lidar_features, image_features, modality_weights) -> np.ndarray:
    out, _ = run(lidar_features, image_features, modality_weights, trace=False)
    return out
